# revision 1
# baseline (speedup 1.0000x reference)
# CapsuleLayer dynamic-routing kernel for 8x Trainium2 NeuronCores.
#
# Problem: u_hat[b,n,m,d] = sum_i W[n,m,d,i] * x[b,m,i]; 3 routing iterations
#   c = softmax_n(blog); s[b,n,d] = sum_m c*u_hat; out = squash_d(s);
#   blog += sum_d out*u_hat
# with B=128, M=2048, I=8, N=32, D=16.
#
# Sharding: M (input capsules) split across 8 cores, 256 per core. The
# softmax (over n) and the b-logit update are local to an m-shard; only the
# tiny s[b,n,d] partial sums (256KB) cross cores, via AllReduce, once per
# routing iteration. W never leaves SBUF after the initial load.
#
# Key identities used to keep everything on the PE / fused DVE ops:
#   blog_k[b,n,m] = <Rsum_k[b,n,:], u_hat[b,n,m,:]>  where Rsum_k = sum_{j<k} out_j
#     -> per (n,i):  psi[b,m] = sum_d Rsum[d,b] * W[n,m,d,i]   (PE, K=16)
#        blog[b,n,m] = sum_i psi_i[b,m] * x[b,m,i]             (DVE mult+reduce)
#   s_k via softmax numerator folded into x:
#     xr[m,(i,b)] = xT * (1/sum_n exp)   z_n[m,(i,b)] = expT_n * xr
#     s^T[d,(n,b)] += W2[m,(n,i,d)].T @ z_n                    (PE, K=128)
# Layouts are host-prepared so every DMA is contiguous or coarsely strided.

import numpy as np

import concourse.bacc as bacc
import concourse.mybir as mybir
import concourse.tile as tile
from concourse.bass_utils import run_bass_kernel_spmd

B = 128          # batch (== SBUF partitions)
MTOT = 2048      # input capsules
I = 8            # input capsule dim
N = 32           # output capsules
D = 16           # output capsule dim
CORES = 8
MC = MTOT // CORES   # 256 input capsules per core
CH = 2               # m chunks of 128 per core
MCH = MC // CH       # 128
ND = N * D           # 512
EPS = 1e-7
ROUTINGS = 3

F32 = mybir.dt.float32
F32R = mybir.dt.float32r
ADD = mybir.AluOpType.add
MULT = mybir.AluOpType.mult
AX_X = mybir.AxisListType.X
ACT = mybir.ActivationFunctionType

DEBUG = False

_CACHE = {}


def _build_nc(debug_outputs: bool, no_collective: bool = False):
    nc = bacc.Bacc("TRN2", target_bir_lowering=False, debug=False,
                   num_devices=1 if no_collective else CORES)

    xbm_d = nc.dram_tensor("xbm", [B, MC * I], F32, kind="ExternalInput").ap()
    xt_d = nc.dram_tensor("xt", [MC, I * B], F32R, kind="ExternalInput").ap()
    wpsi_d = nc.dram_tensor("wpsi", [I * D, N * MC], F32R, kind="ExternalInput").ap()
    w2_d = nc.dram_tensor("w2", [MC, N * I * D], F32R, kind="ExternalInput").ap()
    ident_d = nc.dram_tensor("ident", [128, 128], F32, kind="ExternalInput").ap()
    bones_d = nc.dram_tensor("bones", [128, 128], F32, kind="ExternalInput").ap()
    out_d = nc.dram_tensor("out_f", [128, ND], F32, kind="ExternalOutput").ap()

    dbg = {}
    if debug_outputs:
        for name, shape in [
            ("dbg_s0", [128, ND]),
            ("dbg_o0", [128, ND]),
            ("dbg_rt1", [128, N * B]),
            ("dbg_b1", [B, N * MC]),
        ]:
            dbg[name] = nc.dram_tensor(name, shape, F32, kind="ExternalOutput").ap()

    with tile.TileContext(nc) as tc:
        with tc.tile_pool(name="const", bufs=1) as cp, \
             tc.tile_pool(name="work", bufs=1) as wp, \
             tc.tile_pool(name="psum", bufs=1, space="PSUM") as pp, \
             tc.tile_pool(name="dram", bufs=1, space="DRAM") as dp:

            # ---- persistent SBUF ----
            xbm = cp.tile([B, MC * I], F32, tag="xbm")
            xts = [cp.tile([MCH, I * B], F32R, tag=f"xt{c}", name=f"xt{c}") for c in range(CH)]
            wpsi = cp.tile([I * D, N * MC], F32R, tag="wpsi")
            w2s = [cp.tile([MCH, N * I * D], F32R, tag=f"w2{c}", name=f"w2{c}") for c in range(CH)]
            ident = cp.tile([128, 128], F32, tag="ident")
            eps_t = cp.tile([128, 1], F32, tag="eps")
            bones = cp.tile([128, 128], F32, tag="bones")
            rtA = wp.tile([128, N * B], F32R, tag="rtA")
            rtB = wp.tile([128, N * B], F32R, tag="rtB")
            osA = wp.tile([128, ND], F32, tag="osA")
            osB = wp.tile([128, ND], F32, tag="osB")

            nc.sync.dma_start(xbm[:], xbm_d)
            for c in range(CH):
                nc.sync.dma_start(xts[c][:], xt_d[c * MCH:(c + 1) * MCH, :])
            nc.sync.dma_start(wpsi[:], wpsi_d)
            for c in range(CH):
                nc.sync.dma_start(w2s[c][:], w2_d[c * MCH:(c + 1) * MCH, :])
            nc.sync.dma_start(ident[:], ident_d)
            nc.gpsimd.memset(eps_t[:], EPS)
            nc.gpsimd.memset(rtA[:].bitcast(F32), 0.0)
            nc.gpsimd.memset(rtB[:].bitcast(F32), 0.0)
            nc.sync.dma_start(bones[:], bones_d)

            for k in range(ROUTINGS):
                # ---------- logits + transposed exp (passes 1,2) ----------
                if k > 0:
                    if debug_outputs and k == 1:
                        nc.sync.dma_start(dbg["dbg_rt1"], rtA[:].bitcast(F32))
                    # expT[(m), c*4096 + n*128 + b] = exp(blog[b, n, m])
                    expT = wp.tile([MCH, CH * N * B], F32, tag="expT")
                    for n in range(N):
                        psi = pp.tile([B, MC * I], F32, tag="psi", bufs=1)
                        for i in range(I):
                            q = i // 2
                            rt_src = rtA if i % 2 == 0 else rtB
                            nc.tensor.matmul(
                                psi[:, i * MC:(i + 1) * MC],
                                lhsT=rt_src[32 * q:32 * q + 32, n * B:(n + 1) * B],
                                rhs=wpsi[32 * q:32 * q + 32, n * MC:(n + 1) * MC],
                                start=True, stop=True,
                                tile_position=(32 * q, 0),
                            )
                        tmp = wp.tile([B, MC * I], F32, tag="tmp", bufs=1)
                        nc.vector.tensor_tensor(
                            tmp.rearrange("p (m i) -> p m i", i=I),
                            psi.rearrange("p (i m) -> p m i", i=I),
                            xbm.rearrange("p (m i) -> p m i", i=I),
                            MULT,
                        )
                        bun = wp.tile([B, MC], F32, tag="bun", bufs=2)
                        nc.vector.reduce_sum(
                            bun[:],
                            tmp.rearrange("p (m i) -> p m i", i=I),
                            axis=AX_X,
                        )
                        if debug_outputs and k == 1:
                            nc.sync.dma_start(dbg["dbg_b1"][:, n * MC:(n + 1) * MC], bun[:])
                        for c in range(CH):
                            bt = pp.tile([128, 128], F32, tag="bt", bufs=2)
                            nc.tensor.transpose(
                                bt[:], bun[:, c * MCH:(c + 1) * MCH], ident[:])
                            nc.scalar.activation(
                                expT[:, c * (N * B) + n * B: c * (N * B) + (n + 1) * B],
                                bt[:], ACT.Exp,
                            )
                    xrs = []
                    for c in range(CH):
                        den = wp.tile([MCH, B], F32, tag="den", bufs=2)
                        nc.vector.reduce_sum(
                            den[:],
                            expT[:, c * (N * B):(c + 1) * (N * B)].rearrange(
                                "p (n b) -> p b n", n=N),
                            axis=AX_X,
                        )
                        rden = wp.tile([MCH, B], F32, tag="rden", bufs=2)
                        nc.vector.reciprocal(rden[:], den[:])
                        xr = wp.tile([MCH, I * B], F32, tag="xr", bufs=2)
                        nc.vector.tensor_tensor(
                            xr.rearrange("p (i b) -> p i b", i=I),
                            xts[c].bitcast(F32).rearrange("p (i b) -> p i b", i=I),
                            rden.unsqueeze(1).broadcast_to([MCH, I, B]),
                            MULT,
                        )
                        xrs.append(xr)

                # ---------- s^T accumulation ----------
                if k == 0:
                    # pass 0: rhs (xt) is n-independent, so pack 4 n's into
                    # the stationary side (M_out = 64) -> 4x fewer matmuls.
                    sT_sb0 = wp.tile([64, 8 * B], F32, tag="sTsb0", bufs=1)
                    for nq in range(8):
                        sq4 = pp.tile([64, B], F32, tag="ps_small", bufs=2)
                        for c in range(CH):
                            for i in range(I):
                                nc.tensor.matmul(
                                    sq4[0:64, 0:B],
                                    lhsT=w2s[c][:, i * (N * D) + nq * 4 * D:
                                                i * (N * D) + (nq + 1) * 4 * D],
                                    rhs=xts[c][:, i * B:(i + 1) * B],
                                    start=(c == 0 and i == 0),
                                    stop=(c == CH - 1 and i == I - 1),
                                )
                        nc.scalar.copy(sT_sb0[:, nq * B:(nq + 1) * B], sq4[0:64, 0:B])
                    s_in0 = dp.tile([64, 8 * B], F32, tag="sin0", bufs=1)
                    s_out0 = dp.tile([64, 8 * B], F32, tag="sout0", bufs=1)
                    nc.sync.dma_start(s_in0[:], sT_sb0[:])
                    if no_collective:
                        nc.sync.dma_start(s_out0[:], s_in0[:])
                    else:
                        nc.gpsimd.collective_compute(
                            "AllReduce", ADD,
                            replica_groups=[list(range(CORES))],
                            ins=[s_in0.opt()],
                            outs=[s_out0.opt()],
                        )
                    # readback into the shared squash layout ssq[(bh,d),(n,bl)]
                    ssq = wp.tile([128, ND], F32, tag="ssq", bufs=1)
                    for n4 in range(4):
                        src = (s_out0[n4 * 16:(n4 + 1) * 16, :]
                               .rearrange("d (nq bh bl) -> d nq bh bl", nq=8, bh=8, bl=16)
                               .transpose([2, 0, 1, 3]))
                        dst = ssq.rearrange("p (nq n4 bl) -> p nq n4 bl", n4=4, bl=16)[:, :, n4, :]
                        nc.sync.dma_start(dst, src)
                else:
                    sT_sb = wp.tile([D, N * B], F32, tag="sTsb", bufs=1)
                    for n in range(N):
                        sTn = pp.tile([D, B], F32, tag="ps_small", bufs=2)
                        for c in range(CH):
                            z = wp.tile([MCH, I * B], F32R, tag="z", bufs=2)
                            nc.vector.tensor_tensor(
                                z.rearrange("p (i b) -> p i b", i=I),
                                expT[:, c * (N * B) + n * B: c * (N * B) + (n + 1) * B]
                                    .unsqueeze(1).broadcast_to([MCH, I, B]),
                                xrs[c].rearrange("p (i b) -> p i b", i=I),
                                MULT,
                            )
                            for i in range(I):
                                nc.tensor.matmul(
                                    sTn[0:D, 0:B],
                                    lhsT=w2s[c][:, i * (N * D) + n * D: i * (N * D) + (n + 1) * D],
                                    rhs=z[:, i * B:(i + 1) * B],
                                    start=(c == 0 and i == 0),
                                    stop=(c == CH - 1 and i == I - 1),
                                )
                        nc.scalar.copy(sT_sb[:, n * B:(n + 1) * B], sTn[0:D, 0:B])

                    # ---------- AllReduce of s^T partials ----------
                    s_in = dp.tile([D, N * B], F32, tag="sin", bufs=2)
                    s_out = dp.tile([D, N * B], F32, tag="sout", bufs=2)
                    nc.sync.dma_start(s_in[:], sT_sb[:])
                    if no_collective:
                        nc.sync.dma_start(s_out[:], s_in[:])
                    else:
                        nc.gpsimd.collective_compute(
                            "AllReduce", ADD,
                            replica_groups=[list(range(CORES))],
                            ins=[s_in.opt()],
                            outs=[s_out.opt()],
                        )
                    # readback in squash layout: ssq[(bh, d), (n, bl)]
                    ssq = wp.tile([128, ND], F32, tag="ssq", bufs=1)
                    nc.sync.dma_start(
                        ssq[:],
                        s_out.rearrange("d (n bh bl) -> d n bh bl", bh=8, bl=16)
                             .transpose([2, 0, 1, 3]),
                    )

                # ---------- squash ----------
                kscale = (1.0 / N) if k == 0 else 1.0
                sq = wp.tile([128, ND], F32, tag="sqz", bufs=2)
                nc.scalar.activation(sq[:], ssq[:], ACT.Square, scale=kscale)
                s2 = pp.tile([128, ND], F32, tag="ps_small", bufs=2)
                nc.tensor.matmul(s2[:], lhsT=bones[:], rhs=sq[:], start=True, stop=True)
                q = wp.tile([128, ND], F32, tag="sqz", bufs=2)
                nc.scalar.activation(q[:], s2[:], ACT.Sqrt, bias=eps_t[:])
                r = wp.tile([128, ND], F32, tag="sqz", bufs=2)
                nc.vector.scalar_tensor_tensor(r[:], s2[:], 1.0, q[:], ADD, MULT)
                w_ = wp.tile([128, ND], F32, tag="sqz", bufs=2)
                nc.vector.reciprocal(w_[:], r[:])
                sc = wp.tile([128, ND], F32, tag="sqz", bufs=2)
                nc.vector.tensor_tensor(sc[:], s2[:], w_[:], MULT)
                o = wp.tile([128, ND], F32, tag="ot", bufs=1)
                nc.vector.scalar_tensor_tensor(o[:], ssq[:], kscale, sc[:], MULT, MULT)

                if debug_outputs and k == 0:
                    nc.sync.dma_start(dbg["dbg_s0"], ssq[:])
                    nc.sync.dma_start(dbg["dbg_o0"], o[:])

                if k == ROUTINGS - 1:
                    nc.sync.dma_start(out_d, o[:])
                else:
                    # Osum accumulation (small, [(bh,d),(n,bl)] layout)
                    if k == 0:
                        nc.vector.tensor_copy(osA[:], o[:])
                        osum_cur = osA
                    else:
                        nc.vector.tensor_tensor(osB[:], osA[:], o[:], ADD)
                        osum_cur = osB
                    # rebuild Rt[(i,d), (n,b)] = Osum[b,n,d], replicated over i
                    osum_dram = dp.tile([8, D, N, 16], F32R, tag="osd", bufs=2)
                    nc.gpsimd.dma_start(
                        osum_dram.rearrange("bh d n bl -> (bh d) (n bl)"),
                        osum_cur[:],
                    )
                    src = osum_dram.transpose([1, 2, 0, 3])  # [d, n, bh, bl]
                    for i in range(I):
                        q, par = i // 2, i % 2
                        rt_dst = rtA if par == 0 else rtB
                        row0 = 32 * q + 16 * par
                        nc.sync.dma_start(
                            rt_dst[row0:row0 + 16, :].rearrange(
                                "p (n bh bl) -> p n bh bl", bh=8, bl=16),
                            src,
                        )

    nc.compile()
    return nc


def _host_prep(inputs: np.ndarray, W: np.ndarray):
    """Build the per-core input maps (all layouts host-side)."""
    inputs = np.ascontiguousarray(inputs, dtype=np.float32)
    W = np.ascontiguousarray(W, dtype=np.float32)
    ident = np.eye(128, dtype=np.float32)
    bones = np.kron(np.eye(8, dtype=np.float32), np.ones((16, 16), dtype=np.float32))
    in_maps = []
    for c in range(CORES):
        xc = inputs[:, c * MC:(c + 1) * MC, :]            # [B, MC, I]
        Wc = W[:, c * MC:(c + 1) * MC, :, :]              # [N, MC, D, I]
        xbm = np.ascontiguousarray(xc.reshape(B, MC * I))
        xt = np.ascontiguousarray(xc.transpose(1, 2, 0).reshape(MC, I * B))
        wpsi = np.ascontiguousarray(Wc.transpose(3, 2, 0, 1).reshape(I * D, N * MC))
        w2 = np.ascontiguousarray(Wc.transpose(1, 3, 0, 2).reshape(MC, N * I * D))
        in_maps.append({
            "xbm": xbm, "xt": xt, "wpsi": wpsi, "w2": w2,
            "ident": ident, "bones": bones,
        })
    return in_maps


def _decode_out(out_f: np.ndarray) -> np.ndarray:
    # out_f [128, 512] in [(bh, d), (n, bl)] layout -> [b, n, d]
    arr = out_f.reshape(8, D, N, 16)          # bh, d, n, bl
    return np.ascontiguousarray(arr.transpose(0, 3, 2, 1).reshape(B, N, D))


def run(inputs: np.ndarray, W: np.ndarray, trace: bool = False):
    key = ("nc", DEBUG)
    if key not in _CACHE:
        _CACHE[key] = _build_nc(DEBUG)
    nc = _CACHE[key]
    in_maps = _host_prep(inputs, W)
    res = run_bass_kernel_spmd(nc, in_maps, core_ids=list(range(CORES)), trace=trace)
    out = _decode_out(res.results[0]["out_f"])
    return out, res


def kernel(inputs: np.ndarray, W: np.ndarray) -> np.ndarray:
    out, _ = run(inputs, W, trace=False)
    return out



# revision 16
# speedup vs baseline: 2.0672x; 2.0672x over previous
# CapsuleLayer dynamic-routing kernel for 8x Trainium2 NeuronCores — v2.
#
# Problem: u_hat[b,n,m,d] = sum_i W[n,m,d,i] * x[b,m,i]; 3 routing iterations
#   c = softmax_n(blog); s[b,n,d] = sum_m c*u_hat; out = squash_d(s);
#   blog += sum_d out*u_hat
# with B=128, M=2048, I=8, N=32, D=16.
#
# Sharding: M (input capsules) split across 8 cores, 256 per core; only the
# small s[b,n,d] partial sums cross cores (AllReduce) once per iteration.
#
# v2 layout: m on SBUF partitions (two chunks of 128), fp16 compute tensors.
# Per routing pass k>0:
#   phi_{n,i}[m,b] = sum_d W[n,m,d,i]*Rsum[b,n,d]      (PE, K=32 masked-pair)
#   tmp  = phi (PSUM->SBUF f16 copy on Act)
#   tmp2 = tmp * xT                                     (DVE, fp16 2x mode)
#   blogT_n[m,b] = sum_i tmp2                           (DVE tree-add, 2x)
#   expT = exp(blogT)  [Act];  Z = sum_n expT  [DVE tree];  xr = xT / Z
#   z_n[m,(i,b)] = expT_n * xr                          (DVE 2x)
#   sT[(j,d),b] += w0[m,(i,n,d)]^T @ z_n                (PE fp16, PSUM acc)
# s AllReduce in [(j,d),(q,b)] layout (n = q*8+j), squash in-place, Rsum^T
# rebuilt via a DRAM round trip into the zero-masked rtA/rtB pair tiles.

import numpy as np

import concourse.bacc as bacc
import concourse.mybir as mybir
import concourse.tile as tile
from concourse.bass_utils import run_bass_kernel_spmd

B = 128          # batch (== SBUF partitions)
MTOT = 2048      # input capsules
I = 8            # input capsule dim
N = 32           # output capsules
D = 16           # output capsule dim
CORES = 8
MC = MTOT // CORES   # 256 input capsules per core
CH = 2               # m chunks of 128 per core
MCH = MC // CH       # 128
ND = N * D           # 512
EPS = 1e-7
ROUTINGS = 3

F32 = mybir.dt.float32
F16 = mybir.dt.float16
ADD = mybir.AluOpType.add
MULT = mybir.AluOpType.mult
AX_X = mybir.AxisListType.X
ACT = mybir.ActivationFunctionType

_CACHE = {}


def _build_nc(debug_outputs: bool = False, no_collective: bool = False, stage: int = 3):
    nc = bacc.Bacc("TRN2", target_bir_lowering=False, debug=False,
                   num_devices=1 if no_collective else CORES)

    xt_d = nc.dram_tensor("xt", [MCH, CH * I * B], F16, kind="ExternalInput").ap()
    xt2_d = nc.dram_tensor("xt2", [MCH, CH * B * I], F16, kind="ExternalInput").ap()
    wphi_d = nc.dram_tensor("wphi", [128, N * CH * MCH], F16, kind="ExternalInput").ap()
    w0_d = nc.dram_tensor("w0", [MCH, CH * I * N * D], F16, kind="ExternalInput").ap()
    bones_d = nc.dram_tensor("bones", [128, 128], F32, kind="ExternalInput").ap()
    out_d = nc.dram_tensor("out_f", [128, ND], F32, kind="ExternalOutput").ap()

    with tile.TileContext(nc) as tc:
        with tc.tile_pool(name="const", bufs=1) as cp, \
             tc.tile_pool(name="work", bufs=1) as wp, \
             tc.tile_pool(name="tmp4", bufs=2) as tp, \
             tc.tile_pool(name="zp", bufs=3) as zp, \
             tc.tile_pool(name="phip", bufs=2, space="PSUM") as pp, \
             tc.tile_pool(name="dram", bufs=2, space="DRAM") as dp:

            # ---- persistent SBUF ----
            xts = [cp.tile([MCH, I * B], F16, tag=f"xt{c}", name=f"xt{c}")
                   for c in range(CH)]
            xt2s = [cp.tile([MCH, B * I], F16, tag=f"xt2{c}", name=f"xt2{c}")
                    for c in range(CH)]
            wphi = cp.tile([128, N * CH * MCH], F16, tag="wphi")
            w0s = [cp.tile([MCH, I * N * D], F16, tag=f"w0{c}", name=f"w0{c}")
                   for c in range(CH)]
            bones = cp.tile([128, 128], F32, tag="bones")
            eps_t = cp.tile([128, 1], F32, tag="eps")
            shf_t = cp.tile([128, 1], F32, tag="shf")
            rtA = wp.tile([128, N * B], F16, tag="rtA")
            rtB = wp.tile([128, N * B], F16, tag="rtB")
            osum = wp.tile([128, ND], F16, tag="osum")
            blogT = [wp.tile([MCH, N * B], F16, tag=f"blogT{c}", name=f"blogT{c}")
                     for c in range(CH)]
            expT = [wp.tile([MCH, N * B], F16, tag=f"expT{c}", name=f"expT{c}")
                    for c in range(CH)]
            xrs = [wp.tile([MCH, I * B], F16, tag=f"xr{c}", name=f"xr{c}")
                   for c in range(CH)]

            for c in range(CH):
                nc.sync.dma_start(xts[c][:], xt_d[:, c * (I * B):(c + 1) * (I * B)])
                nc.sync.dma_start(xt2s[c][:], xt2_d[:, c * (B * I):(c + 1) * (B * I)])
                nc.sync.dma_start(w0s[c][:], w0_d[:, c * (I * N * D):(c + 1) * (I * N * D)])
            nc.sync.dma_start(bones[:], bones_d)
            nc.sync.dma_start(wphi[:], wphi_d)
            nc.gpsimd.memset(eps_t[:], EPS)
            nc.gpsimd.memset(shf_t[:], -4.0)
            nc.gpsimd.memset(rtA[:].bitcast(F32), 0.0)
            nc.gpsimd.memset(rtB[:].bitcast(F32), 0.0)

            if stage == 1:
                ks = [0]
            elif stage in (15, 2):
                ks = [0, 1]
            else:
                ks = [0, 1, 2]
            last_full = 0 if stage in (1, 15) else ks[-1]
            for k in ks:
                # ---------- logits -> expT (k > 0) ----------
                if k > 0:
                    for c in range(CH):
                        for n4 in range(N // 4):
                            tmp4 = tp.tile([MCH, 4 * B * I], F16, tag="tmp4", name="tmp4")
                            for nn in range(4):
                                n = n4 * 4 + nn
                                phi = pp.tile([MCH, 2048], F32, tag="phi", name="phi")
                                for i in range(I):
                                    q, par = i // 2, i % 2
                                    rt_src = rtA if par == 0 else rtB
                                    off = q * 512 + par * B
                                    nc.tensor.matmul(
                                        phi[:, off:off + B],
                                        lhsT=wphi[32 * q:32 * q + 32,
                                                  (n * CH + c) * MCH:(n * CH + c + 1) * MCH],
                                        rhs=rt_src[32 * q:32 * q + 32, n * B:(n + 1) * B],
                                        start=True, stop=True,
                                        tile_position=(32 * q, 0),
                                    )
                                # PSUM f32 -> SBUF f16, (q,par,b) -> (b, i=2q+par)
                                nc.scalar.copy(
                                    tmp4.rearrange("p (nn b q par) -> p nn q par b",
                                                   nn=4, b=B, q=4, par=2)[:, nn],
                                    phi.rearrange("p (q c4 b) -> p q c4 b",
                                                  q=4, c4=4)[:, :, 0:2, :],
                                )
                            # tmp2 = tmp4 * xT (broadcast over the 4 n's)
                            tmp2 = tp.tile([MCH, 4 * B * I], F16, tag="tmp2", name="tmp2")
                            nc.vector.tensor_tensor(
                                tmp2.rearrange("p (nn bi) -> p nn bi", nn=4),
                                tmp4.rearrange("p (nn bi) -> p nn bi", nn=4),
                                xt2s[c].unsqueeze(1).broadcast_to([MCH, 4, B * I]),
                                MULT,
                            )
                            # tree-reduce over i (innermost, packed)
                            t1 = tp.tile([MCH, 4 * B * 4], F16, tag="t1", name="t1")
                            v = tmp2.rearrange("p (nb i) -> p nb i", i=I)
                            nc.vector.tensor_tensor(
                                t1.rearrange("p (nb i) -> p nb i", i=4),
                                v[:, :, 0:4], v[:, :, 4:8], ADD)
                            t2 = tp.tile([MCH, 4 * B * 2], F16, tag="t2", name="t2")
                            v = t1.rearrange("p (nb i) -> p nb i", i=4)
                            nc.vector.tensor_tensor(
                                t2.rearrange("p (nb i) -> p nb i", i=2),
                                v[:, :, 0:2], v[:, :, 2:4], ADD)
                            v = t2.rearrange("p (nb i) -> p nb i", i=2)
                            nc.vector.tensor_tensor(
                                blogT[c][:, n4 * 4 * B:(n4 + 1) * 4 * B]
                                    .rearrange("p (nb one) -> p nb one", one=1),
                                v[:, :, 0:1], v[:, :, 1:2], ADD)
                        nc.scalar.activation(expT[c][:], blogT[c][:], ACT.Exp, bias=shf_t[:])
                        # Z[m, b] = sum_n expT  (tree over n-blocks)
                        d1 = wp.tile([MCH, 16 * B], F16, tag="d1", name="d1", bufs=1)
                        nc.vector.tensor_tensor(
                            d1[:], expT[c][:, 0:16 * B], expT[c][:, 16 * B:32 * B], ADD)
                        d2 = wp.tile([MCH, 8 * B], F16, tag="d2", name="d2", bufs=1)
                        nc.vector.tensor_tensor(
                            d2[:], d1[:, 0:8 * B], d1[:, 8 * B:16 * B], ADD)
                        d3 = wp.tile([MCH, 4 * B], F16, tag="d3", name="d3", bufs=1)
                        nc.vector.tensor_tensor(
                            d3[:], d2[:, 0:4 * B], d2[:, 4 * B:8 * B], ADD)
                        d4 = wp.tile([MCH, 2 * B], F16, tag="d4", name="d4", bufs=1)
                        nc.vector.tensor_tensor(
                            d4[:], d3[:, 0:2 * B], d3[:, 2 * B:4 * B], ADD)
                        zden = wp.tile([MCH, B], F32, tag="zden", name="zden", bufs=2)
                        nc.vector.tensor_tensor(
                            zden[:], d4[:, 0:B], d4[:, B:2 * B], ADD)
                        rden = wp.tile([MCH, B], F16, tag="rden", name="rden", bufs=2)
                        with nc.allow_low_precision(reason="routing weights tolerate f16"):
                            nc.vector.reciprocal(rden[:], zden[:])
                        nc.vector.tensor_tensor(
                            xrs[c].rearrange("p (i b) -> p i b", i=I),
                            xts[c].rearrange("p (i b) -> p i b", i=I),
                            rden.unsqueeze(1).broadcast_to([MCH, I, B]),
                            MULT,
                        )

                if stage == 15 and k == 1:
                    continue
                # ---------- s^T accumulation ----------
                sT_sb = wp.tile([128, ND], F32, tag="sTsb", bufs=1)
                if k == 0:
                    # uniform c: rhs (xt) is n-independent -> pack 8 n's in
                    # the stationary: lhsT [m, (n8, d)] -> out [(j,d), b]
                    for g in range(N // 8):
                        sacc8_t = pp.tile([MCH, 2048], F32, tag="phi", name="sacc8")
                        sacc8 = sacc8_t
                        for c in range(CH):
                            for i in range(I):
                                nc.tensor.matmul(
                                    sacc8[0:128, 0:B],
                                    lhsT=w0s[c][:, i * (N * D) + g * 8 * D:
                                                i * (N * D) + (g + 1) * 8 * D],
                                    rhs=xts[c][:, i * B:(i + 1) * B],
                                    start=(c == 0 and i == 0),
                                    stop=(c == CH - 1 and i == I - 1),
                                )
                        nc.scalar.copy(sT_sb[:, g * B:(g + 1) * B], sacc8[0:128, 0:B])
                else:
                    sT16 = wp.tile([16, N * B], F32, tag="sT16", bufs=1)
                    for n in range(N):
                        sacc_t = pp.tile([MCH, 2048], F32, tag="phi", name="sacc")
                        sacc = sacc_t
                        for c in range(CH):
                            zn = zp.tile([MCH, I * B], F16, tag="zn", name="zn")
                            nc.vector.tensor_tensor(
                                zn.rearrange("p (i b) -> p i b", i=I),
                                xrs[c].rearrange("p (i b) -> p i b", i=I),
                                expT[c][:, n * B:(n + 1) * B]
                                    .unsqueeze(1).broadcast_to([MCH, I, B]),
                                MULT,
                            )
                            for i in range(I):
                                nc.tensor.matmul(
                                    sacc[0:D, 0:B],
                                    lhsT=w0s[c][:, i * (N * D) + n * D:
                                                i * (N * D) + (n + 1) * D],
                                    rhs=zn[:, i * B:(i + 1) * B],
                                    start=(c == 0 and i == 0),
                                    stop=(c == CH - 1 and i == I - 1),
                                )
                        nc.scalar.copy(sT16[0:16, n * B:(n + 1) * B], sacc[0:D, 0:B])

                # ---------- AllReduce of s^T partials ----------
                # ssq ends up [(j, d), (q, b)] with n = q*8 + j for every k.
                ssq = wp.tile([128, ND], F32, tag="ssq", bufs=1)
                if k == 0:
                    s_in = dp.tile([128, ND], F32, tag="sin", bufs=2)
                    s_out = dp.tile([128, ND], F32, tag="sout", bufs=2)
                    nc.sync.dma_start(s_in[:], sT_sb[:])
                    if no_collective:
                        nc.sync.dma_start(s_out[:], s_in[:])
                    else:
                        nc.gpsimd.collective_compute(
                            "AllReduce", ADD,
                            replica_groups=[list(range(CORES))],
                            ins=[s_in.opt()],
                            outs=[s_out.opt()],
                        )
                    nc.sync.dma_start(ssq[:], s_out[:])
                else:
                    s_in1 = dp.tile([16, N * B], F32, tag="sin1", bufs=2)
                    s_out1 = dp.tile([16, N * B], F32, tag="sout1", bufs=2)
                    nc.sync.dma_start(s_in1[:], sT16[:])
                    if no_collective:
                        nc.sync.dma_start(s_out1[:], s_in1[:])
                    else:
                        nc.gpsimd.collective_compute(
                            "AllReduce", ADD,
                            replica_groups=[list(range(CORES))],
                            ins=[s_in1.opt()],
                            outs=[s_out1.opt()],
                        )
                    # scatter [d, (q, j, b)] -> [(j, d), (q, b)] during readback
                    nc.sync.dma_start(
                        ssq[:],
                        s_out1.rearrange("d (q j b) -> j d q b", q=4, j=8),
                    )

                # ---------- squash (layout [(j,d), (q,b)], n = q*8+j) ----------
                kscale = (1.0 / N) if k == 0 else 1.0
                sq = wp.tile([128, ND], F32, tag="sqz", bufs=2)
                nc.scalar.activation(sq[:], ssq[:], ACT.Square, scale=kscale)
                s2_t = pp.tile([MCH, 2048], F32, tag="phi", name="ps_sq")
                s2 = s2_t[:, 0:ND]
                nc.tensor.matmul(s2, lhsT=bones[:], rhs=sq[:], start=True, stop=True)
                qq = wp.tile([128, ND], F32, tag="sqz", bufs=2)
                nc.scalar.activation(qq[:], s2, ACT.Sqrt, bias=eps_t[:])
                rr = wp.tile([128, ND], F32, tag="sqz", bufs=2)
                nc.vector.scalar_tensor_tensor(rr[:], s2, 1.0, qq[:], ADD, MULT)
                ww = wp.tile([128, ND], F32, tag="sqz", bufs=2)
                nc.vector.reciprocal(ww[:], rr[:])
                sc = wp.tile([128, ND], F32, tag="sqz", bufs=2)
                nc.vector.tensor_tensor(sc[:], s2, ww[:], MULT)
                o = wp.tile([128, ND], F16 if k < last_full else F32,
                            tag="ot" if k < last_full else "ot32", bufs=2)
                with nc.allow_low_precision(reason="outputs tolerate f16"):
                    nc.vector.scalar_tensor_tensor(o[:], ssq[:], kscale, sc[:], MULT, MULT)

                if k == last_full:
                    nc.sync.dma_start(out_d, o[:])
                if k < ks[-1]:
                    if k == 0:
                        nc.vector.tensor_copy(osum[:], o[:])
                    else:
                        nc.vector.tensor_tensor(osum[:], osum[:], o[:], ADD)
                    # Rsum^T rebuild: osum [(j,d),(q,b)] f16 -> DRAM in
                    # [d, (n,b)] layout (scatter on store), then contiguous
                    # replicating loads into the rt pair tiles.
                    o_dram = dp.tile([D, N * B], F16, tag="osd", bufs=2)
                    nc.sync.dma_start(
                        o_dram.rearrange("d (q j b) -> j d q b", q=4, j=8),
                        osum[:])
                    # rt rows 32q'+16par+d hold RsumT[d, (n,b)]
                    for qq_ in range(4):
                        for par in range(2):
                            rt_dst = rtA if par == 0 else rtB
                            row0 = 32 * qq_ + 16 * par
                            nc.sync.dma_start(rt_dst[row0:row0 + 16, :], o_dram[:, :])

    nc.compile()
    return nc


def _host_prep(inputs: np.ndarray, W: np.ndarray):
    """Build the per-core input maps (all layouts host-side)."""
    inputs = np.ascontiguousarray(inputs, dtype=np.float32)
    W = np.ascontiguousarray(W, dtype=np.float32)
    bones = np.kron(np.eye(8, dtype=np.float32),
                    np.ones((16, 16), dtype=np.float32))
    in_maps = []
    for core in range(CORES):
        xc = inputs[:, core * MC:(core + 1) * MC, :]      # [B, MC, I]
        Wc = W[:, core * MC:(core + 1) * MC, :, :]        # [N, MC, D, I]
        # xt[m, (c, i, b)]: per chunk, (i, b) layout
        xcr = xc.reshape(B, CH, MCH, I)
        xt = xcr.transpose(2, 1, 3, 0).reshape(MCH, CH * I * B)
        # xt2[m, (c, b, i)]
        xt2 = xcr.transpose(2, 1, 0, 3).reshape(MCH, CH * B * I)
        # wphi[32q+16par+d, (n, c, m)] = W[n, m, d, i], i = 2q+par
        Wr = Wc.reshape(N, CH, MCH, D, I)
        wphi = np.zeros((4, 2, D, N, CH, MCH), dtype=np.float32)
        for i in range(I):
            q, par = i // 2, i % 2
            wphi[q, par] = Wr[:, :, :, :, i].transpose(3, 0, 1, 2)
        wphi = wphi.reshape(128, N * CH * MCH)
        # w0[m, (c, i, n, d)]
        w0 = Wr.transpose(2, 1, 4, 0, 3).reshape(MCH, CH * I * N * D)
        in_maps.append({
            "xt": np.ascontiguousarray(xt, dtype=np.float16),
            "xt2": np.ascontiguousarray(xt2, dtype=np.float16),
            "wphi": np.ascontiguousarray(wphi, dtype=np.float16),
            "w0": np.ascontiguousarray(w0, dtype=np.float16),
            "bones": bones,
        })
    return in_maps


def _decode_out(out_f: np.ndarray) -> np.ndarray:
    # out_f [128, 512] in [(j, d), (q, b)] layout, n = q*8+j -> [b, n, d]
    arr = out_f.astype(np.float32).reshape(8, D, 4, B)    # j, d, q, b
    return np.ascontiguousarray(
        arr.transpose(3, 2, 0, 1).reshape(B, N, D))


def run(inputs: np.ndarray, W: np.ndarray, trace: bool = False):
    key = "nc"
    if key not in _CACHE:
        _CACHE[key] = _build_nc(False)
    nc = _CACHE[key]
    in_maps = _host_prep(inputs, W)
    res = run_bass_kernel_spmd(nc, in_maps, core_ids=list(range(CORES)), trace=trace)
    out = _decode_out(res.results[0]["out_f"])
    return out, res


def kernel(inputs: np.ndarray, W: np.ndarray) -> np.ndarray:
    out, _ = run(inputs, W, trace=False)
    return out


# revision 34
# speedup vs baseline: 2.0892x; 1.0106x over previous
# CapsuleLayer dynamic-routing kernel for 8x Trainium2 NeuronCores — v2.
#
# Problem: u_hat[b,n,m,d] = sum_i W[n,m,d,i] * x[b,m,i]; 3 routing iterations
#   c = softmax_n(blog); s[b,n,d] = sum_m c*u_hat; out = squash_d(s);
#   blog += sum_d out*u_hat
# with B=128, M=2048, I=8, N=32, D=16.
#
# Sharding: M (input capsules) split across 8 cores, 256 per core; only the
# small s[b,n,d] partial sums cross cores (AllReduce) once per iteration.
#
# v2 layout: m on SBUF partitions (two chunks of 128), fp16 compute tensors.
# Per routing pass k>0:
#   phi_{n,i}[m,b] = sum_d W[n,m,d,i]*Rsum[b,n,d]      (PE, K=32 masked-pair)
#   tmp  = phi (PSUM->SBUF f16 copy on Act)
#   tmp2 = tmp * xT                                     (DVE, fp16 2x mode)
#   blogT_n[m,b] = sum_i tmp2                           (DVE tree-add, 2x)
#   expT = exp(blogT)  [Act];  Z = sum_n expT  [DVE tree];  xr = xT / Z
#   z_n[m,(i,b)] = expT_n * xr                          (DVE 2x)
#   sT[(j,d),b] += w0[m,(i,n,d)]^T @ z_n                (PE fp16, PSUM acc)
# s AllReduce in [(j,d),(q,b)] layout (n = q*8+j), squash in-place, Rsum^T
# rebuilt via a DRAM round trip into the zero-masked rtA/rtB pair tiles.

import numpy as np

import concourse.bacc as bacc
import concourse.mybir as mybir
import concourse.tile as tile
from concourse.bass_utils import run_bass_kernel_spmd

B = 128          # batch (== SBUF partitions)
MTOT = 2048      # input capsules
I = 8            # input capsule dim
N = 32           # output capsules
D = 16           # output capsule dim
CORES = 8
MC = MTOT // CORES   # 256 input capsules per core
CH = 2               # m chunks of 128 per core
MCH = MC // CH       # 128
ND = N * D           # 512
EPS = 1e-7
ROUTINGS = 3

F32 = mybir.dt.float32
F16 = mybir.dt.float16
ADD = mybir.AluOpType.add
MULT = mybir.AluOpType.mult
AX_X = mybir.AxisListType.X
ACT = mybir.ActivationFunctionType

_CACHE = {}


def _build_nc(debug_outputs: bool = False, no_collective: bool = False, stage: int = 3):
    nc = bacc.Bacc("TRN2", target_bir_lowering=False, debug=False,
                   num_devices=1 if no_collective else CORES)

    xt_d = nc.dram_tensor("xt", [MCH, CH * I * B], F16, kind="ExternalInput").ap()
    xt2_d = nc.dram_tensor("xt2", [MCH, CH * 2048], F16, kind="ExternalInput").ap()
    wphi_d = nc.dram_tensor("wphi", [128, N * CH * MCH], F16, kind="ExternalInput").ap()
    w0_d = nc.dram_tensor("w0", [MCH, CH * I * N * D], F16, kind="ExternalInput").ap()
    bones_d = nc.dram_tensor("bones", [128, 128], F16, kind="ExternalInput").ap()
    out_d = nc.dram_tensor("out_f", [128, ND], F32, kind="ExternalOutput").ap()

    with tile.TileContext(nc) as tc:
        with tc.tile_pool(name="const", bufs=1) as cp, \
             tc.tile_pool(name="work", bufs=1) as wp, \
             tc.tile_pool(name="tmp4", bufs=4) as tp, \
             tc.tile_pool(name="zp", bufs=3) as zp, \
             tc.tile_pool(name="phip", bufs=2, space="PSUM") as pp, \
             tc.tile_pool(name="dram", bufs=2, space="DRAM") as dp:

            # ---- persistent SBUF ----
            xts = [cp.tile([MCH, I * B], F16, tag=f"xt{c}", name=f"xt{c}")
                   for c in range(CH)]
            xt2s = [cp.tile([MCH, 2048], F16, tag=f"xt2{c}", name=f"xt2{c}")
                    for c in range(CH)]
            wphi = cp.tile([128, N * CH * MCH], F16, tag="wphi")
            w0s = [cp.tile([MCH, I * N * D], F16, tag=f"w0{c}", name=f"w0{c}")
                   for c in range(CH)]
            bones = cp.tile([128, 128], F16, tag="bones")
            eps_t = cp.tile([128, 1], F32, tag="eps")
            shf_t = cp.tile([128, 1], F32, tag="shf")
            rtA = wp.tile([128, N * B], F16, tag="rtA")
            rtB = wp.tile([128, N * B], F16, tag="rtB")
            osum = wp.tile([128, ND], F16, tag="osum")
            blogT = [wp.tile([MCH, N * B], F16, tag=f"blogT{c}", name=f"blogT{c}")
                     for c in range(CH)]
            expT = [wp.tile([MCH, N * B], F16, tag=f"expT{c}", name=f"expT{c}")
                    for c in range(CH)]
            xrs = [wp.tile([MCH, I * B], F16, tag=f"xr{c}", name=f"xr{c}")
                   for c in range(CH)]

            for c in range(CH):
                nc.sync.dma_start(xts[c][:], xt_d[:, c * (I * B):(c + 1) * (I * B)])
                nc.sync.dma_start(w0s[c][:], w0_d[:, c * (I * N * D):(c + 1) * (I * N * D)])
            nc.sync.dma_start(bones[:], bones_d)
            for c in range(CH):
                nc.sync.dma_start(xt2s[c][:], xt2_d[:, c * 2048:(c + 1) * 2048])
            nc.sync.dma_start(wphi[:], wphi_d)
            nc.gpsimd.memset(eps_t[:], EPS)
            nc.gpsimd.memset(shf_t[:], -4.0)
            nc.gpsimd.memset(rtA[:].bitcast(F32), 0.0)
            nc.gpsimd.memset(rtB[:].bitcast(F32), 0.0)

            if stage == 1:
                ks = [0]
            elif stage in (15, 2):
                ks = [0, 1]
            else:
                ks = [0, 1, 2]
            last_full = 0 if stage in (1, 15) else ks[-1]
            for k in ks:
                # ---------- logits -> expT (k > 0) ----------
                if k > 0:
                    for c in range(CH):
                        for np_ in range(N // 2):
                            n0 = np_ * 2
                            # phi pair: [m, (q, n2, par, b)]; bank q holds only
                            # tile_position q (both n's of the pair).
                            phi = pp.tile([MCH, 2048], F32, tag="phi", name="phi")
                            for n2 in range(2):
                                n = n0 + n2
                                for i in range(I):
                                    q, par = i // 2, i % 2
                                    rt_src = rtA if par == 0 else rtB
                                    off = q * 512 + n2 * 256 + par * B
                                    nc.tensor.matmul(
                                        phi[:, off:off + B],
                                        lhsT=wphi[32 * q:32 * q + 32,
                                                  (n * CH + c) * MCH:(n * CH + c + 1) * MCH],
                                        rhs=rt_src[32 * q:32 * q + 32, n * B:(n + 1) * B],
                                        start=True, stop=True,
                                        tile_position=(32 * q, 0),
                                    )
                            # contiguous PSUM f32 -> SBUF f16 evacuation (2 n's)
                            tmp = tp.tile([MCH, 2048], F16, tag="tmp4", name="tmp")
                            nc.scalar.copy(tmp[:], phi[:])
                            # tmp2 = tmp * x  (x replicated over n2 host-side)
                            tmp2 = tp.tile([MCH, 2048], F16, tag="tmp2", name="tmp2")
                            nc.vector.tensor_tensor(tmp2[:], tmp[:], xt2s[c][:], MULT)
                            # tree-reduce over i = (q, par): q-halves twice, then par
                            v = tmp2.rearrange("p (q r) -> p q r", q=4)
                            t1 = tp.tile([MCH, 1024], F16, tag="t1", name="t1")
                            nc.vector.tensor_tensor(
                                t1.rearrange("p (q r) -> p q r", q=2),
                                v[:, 0:2], v[:, 2:4], ADD)
                            v = t1.rearrange("p (q r) -> p q r", q=2)
                            t2 = tp.tile([MCH, 512], F16, tag="t2", name="t2")
                            nc.vector.tensor_tensor(
                                t2.unsqueeze(1), v[:, 0:1], v[:, 1:2], ADD)
                            v = t2.rearrange("p (n2 par b) -> p n2 par b", n2=2, par=2)
                            nc.vector.tensor_tensor(
                                blogT[c][:, n0 * B:(n0 + 2) * B]
                                    .rearrange("p (n2 b) -> p n2 b", n2=2).unsqueeze(2),
                                v[:, :, 0:1], v[:, :, 1:2], ADD)
                        nc.scalar.activation(expT[c][:], blogT[c][:], ACT.Exp, bias=shf_t[:])
                        # Z[m, b] = sum_n expT (tree over n-blocks, on GPSIMD)
                        d1 = wp.tile([MCH, 16 * B], F16, tag="d1", name="d1", bufs=1)
                        nc.vector.tensor_tensor(
                            d1[:], expT[c][:, 0:16 * B], expT[c][:, 16 * B:32 * B], ADD)
                        d2 = wp.tile([MCH, 8 * B], F16, tag="d2", name="d2", bufs=1)
                        nc.vector.tensor_tensor(
                            d2[:], d1[:, 0:8 * B], d1[:, 8 * B:16 * B], ADD)
                        d3 = wp.tile([MCH, 4 * B], F16, tag="d3", name="d3", bufs=1)
                        nc.vector.tensor_tensor(
                            d3[:], d2[:, 0:4 * B], d2[:, 4 * B:8 * B], ADD)
                        d4 = wp.tile([MCH, 2 * B], F16, tag="d4", name="d4", bufs=1)
                        nc.vector.tensor_tensor(
                            d4[:], d3[:, 0:2 * B], d3[:, 2 * B:4 * B], ADD)
                        zden = wp.tile([MCH, B], F32, tag="zden", name="zden", bufs=2)
                        nc.vector.tensor_tensor(
                            zden[:], d4[:, 0:B], d4[:, B:2 * B], ADD)
                        rden = wp.tile([MCH, B], F16, tag="rden", name="rden", bufs=2)
                        with nc.allow_low_precision(reason="routing weights tolerate f16"):
                            nc.vector.reciprocal(rden[:], zden[:])
                        nc.vector.tensor_tensor(
                            xrs[c].rearrange("p (i b) -> p i b", i=I),
                            xts[c].rearrange("p (i b) -> p i b", i=I),
                            rden.unsqueeze(1).broadcast_to([MCH, I, B]),
                            MULT,
                        )

                if stage == 15 and k == 1:
                    continue
                # ---------- s^T accumulation ----------
                sT_sb = wp.tile([128, ND], F32, tag="sTsb", bufs=1)
                if k == 0:
                    # uniform c: rhs (xt) is n-independent -> pack 8 n's in
                    # the stationary: lhsT [m, (n8, d)] -> out [(j,d), b]
                    for g in range(N // 8):
                        sacc8_t = pp.tile([MCH, 2048], F32, tag="phi", name="sacc8")
                        sacc8 = sacc8_t
                        for c in range(CH):
                            for i in range(I):
                                nc.tensor.matmul(
                                    sacc8[0:128, 0:B],
                                    lhsT=w0s[c][:, i * (N * D) + g * 8 * D:
                                                i * (N * D) + (g + 1) * 8 * D],
                                    rhs=xts[c][:, i * B:(i + 1) * B],
                                    start=(c == 0 and i == 0),
                                    stop=(c == CH - 1 and i == I - 1),
                                )
                        nc.scalar.copy(sT_sb[:, g * B:(g + 1) * B], sacc8[0:128, 0:B])
                else:
                    sT16 = wp.tile([16, N * B], F32, tag="sT16", bufs=1)
                    for np_ in range(N // 2):
                        n0 = np_ * 2
                        sacc_t = pp.tile([MCH, 2048], F32, tag="phi", name="sacc")
                        for n2 in range(2):
                            n = n0 + n2
                            for c in range(CH):
                                zn = zp.tile([MCH, I * B], F16, tag="zn", name="zn")
                                nc.vector.tensor_tensor(
                                    zn.rearrange("p (i b) -> p i b", i=I),
                                    xrs[c].rearrange("p (i b) -> p i b", i=I),
                                    expT[c][:, n * B:(n + 1) * B]
                                        .unsqueeze(1).broadcast_to([MCH, I, B]),
                                    MULT,
                                )
                                for i in range(I):
                                    nc.tensor.matmul(
                                        sacc_t[0:D, n2 * B:(n2 + 1) * B],
                                        lhsT=w0s[c][:, i * (N * D) + n * D:
                                                    i * (N * D) + (n + 1) * D],
                                        rhs=zn[:, i * B:(i + 1) * B],
                                        start=(c == 0 and i == 0),
                                        stop=(c == CH - 1 and i == I - 1),
                                    )
                        nc.scalar.copy(sT16[0:16, n0 * B:(n0 + 2) * B],
                                       sacc_t[0:D, 0:2 * B])
                # ---------- AllReduce of s^T partials ----------
                # ssq ends up [(j, d), (q, b)] with n = q*8 + j for every k.
                ssq = wp.tile([128, ND], F32, tag="ssq", bufs=1)
                if k == 0:
                    s_in = dp.tile([128, ND], F32, tag="sin", bufs=2)
                    s_out = dp.tile([128, ND], F32, tag="sout", bufs=2)
                    nc.sync.dma_start(s_in[:], sT_sb[:])
                    if no_collective:
                        nc.sync.dma_start(s_out[:], s_in[:])
                    else:
                        nc.gpsimd.collective_compute(
                            "AllReduce", ADD,
                            replica_groups=[list(range(CORES))],
                            ins=[s_in.opt()],
                            outs=[s_out.opt()],
                        )
                    nc.sync.dma_start(ssq[:], s_out[:])
                else:
                    s_in1 = dp.tile([16, N * B], F32, tag="sin1", bufs=2)
                    s_out1 = dp.tile([16, N * B], F32, tag="sout1", bufs=2)
                    nc.sync.dma_start(s_in1[:], sT16[:])
                    if no_collective:
                        nc.sync.dma_start(s_out1[:], s_in1[:])
                    else:
                        nc.gpsimd.collective_compute(
                            "AllReduce", ADD,
                            replica_groups=[list(range(CORES))],
                            ins=[s_in1.opt()],
                            outs=[s_out1.opt()],
                        )
                    # scatter [d, (q, j, b)] -> [(j, d), (q, b)] during readback
                    nc.sync.dma_start(
                        ssq[:],
                        s_out1.rearrange("d (q j b) -> j d q b", q=4, j=8),
                    )

                # ---------- squash (layout [(j,d), (q,b)], n = q*8+j) ----------
                kscale = (1.0 / N) if k == 0 else 1.0
                sq = wp.tile([128, ND], F16, tag="sqf", bufs=2)
                nc.scalar.activation(sq[:], ssq[:], ACT.Square, scale=kscale)
                s2_t = pp.tile([MCH, 2048], F32, tag="phi", name="ps_sq")
                s2 = s2_t[:, 0:ND]
                nc.tensor.matmul(s2, lhsT=bones[:], rhs=sq[:], start=True, stop=True)
                qq = wp.tile([128, ND], F32, tag="sqz", bufs=2)
                nc.scalar.activation(qq[:], s2, ACT.Sqrt, bias=eps_t[:])
                rr = wp.tile([128, ND], F32, tag="sqz", bufs=2)
                nc.vector.scalar_tensor_tensor(rr[:], s2, 1.0, qq[:], ADD, MULT)
                ww = wp.tile([128, ND], F32, tag="sqz", bufs=2)
                nc.vector.reciprocal(ww[:], rr[:])
                sc = wp.tile([128, ND], F32, tag="sqz", bufs=2)
                nc.vector.tensor_tensor(sc[:], s2, ww[:], MULT)
                o = wp.tile([128, ND], F16 if k < last_full else F32,
                            tag="ot" if k < last_full else "ot32", bufs=2)
                with nc.allow_low_precision(reason="outputs tolerate f16"):
                    nc.vector.scalar_tensor_tensor(o[:], ssq[:], kscale, sc[:], MULT, MULT)

                if k == last_full:
                    nc.sync.dma_start(out_d, o[:])
                if k < ks[-1]:
                    if k == 0:
                        nc.vector.tensor_copy(osum[:], o[:])
                    else:
                        nc.vector.tensor_tensor(osum[:], osum[:], o[:], ADD)
                    # Rsum^T rebuild: osum [(j,d),(q,b)] f16 -> DRAM in
                    # [d, (n,b)] layout (scatter on store), then contiguous
                    # replicating loads into the rt pair tiles.
                    o_dram = dp.tile([D, N * B], F16, tag="osd", bufs=2)
                    nc.sync.dma_start(
                        o_dram.rearrange("d (q j b) -> j d q b", q=4, j=8),
                        osum[:])
                    # rt rows 32q'+16par+d hold RsumT[d, (n,b)]
                    for qq_ in range(4):
                        for par in range(2):
                            rt_dst = rtA if par == 0 else rtB
                            row0 = 32 * qq_ + 16 * par
                            eng = nc.sync if par == 0 else nc.scalar
                            eng.dma_start(rt_dst[row0:row0 + 16, :], o_dram[:, :])

    nc.compile()
    return nc


def _host_prep(inputs: np.ndarray, W: np.ndarray):
    """Build the per-core input maps (all layouts host-side)."""
    inputs = np.ascontiguousarray(inputs, dtype=np.float32)
    W = np.ascontiguousarray(W, dtype=np.float32)
    bones = np.kron(np.eye(8, dtype=np.float32),
                    np.ones((16, 16), dtype=np.float32))
    in_maps = []
    for core in range(CORES):
        xc = inputs[:, core * MC:(core + 1) * MC, :]      # [B, MC, I]
        Wc = W[:, core * MC:(core + 1) * MC, :, :]        # [N, MC, D, I]
        # xt[m, (c, i, b)]: per chunk, (i, b) layout
        xcr = xc.reshape(B, CH, MCH, I)
        xt = xcr.transpose(2, 1, 3, 0).reshape(MCH, CH * I * B)
        # wphi[32q+16par+d, (n, c, m)] = W[n, m, d, i], i = 2q+par
        Wr = Wc.reshape(N, CH, MCH, D, I)
        wphi = np.zeros((4, 2, D, N, CH, MCH), dtype=np.float32)
        for i in range(I):
            q, par = i // 2, i % 2
            wphi[q, par] = Wr[:, :, :, :, i].transpose(3, 0, 1, 2)
        wphi = wphi.reshape(128, N * CH * MCH)
        # xt2[m, (c, q, n2, par, b)]: x replicated over the n-pair dim
        arr = xcr.transpose(2, 1, 3, 0).reshape(MCH, CH, 4, 2, B)
        xt2 = np.broadcast_to(arr[:, :, :, None, :, :],
                              (MCH, CH, 4, 2, 2, B)).reshape(MCH, CH * 2048)
        # w0[m, (c, i, n, d)]
        w0 = Wr.transpose(2, 1, 4, 0, 3).reshape(MCH, CH * I * N * D)
        in_maps.append({
            "xt": np.ascontiguousarray(xt, dtype=np.float16),
            "xt2": np.ascontiguousarray(xt2, dtype=np.float16),
            "wphi": np.ascontiguousarray(wphi, dtype=np.float16),
            "w0": np.ascontiguousarray(w0, dtype=np.float16),
            "bones": bones.astype(np.float16),
        })
    return in_maps


def _decode_out(out_f: np.ndarray) -> np.ndarray:
    # out_f [128, 512] in [(j, d), (q, b)] layout, n = q*8+j -> [b, n, d]
    arr = out_f.astype(np.float32).reshape(8, D, 4, B)    # j, d, q, b
    return np.ascontiguousarray(
        arr.transpose(3, 2, 0, 1).reshape(B, N, D))


def run(inputs: np.ndarray, W: np.ndarray, trace: bool = False):
    key = "nc"
    if key not in _CACHE:
        _CACHE[key] = _build_nc(False)
    nc = _CACHE[key]
    in_maps = _host_prep(inputs, W)
    res = run_bass_kernel_spmd(nc, in_maps, core_ids=list(range(CORES)), trace=trace)
    out = _decode_out(res.results[0]["out_f"])
    return out, res


def kernel(inputs: np.ndarray, W: np.ndarray) -> np.ndarray:
    out, _ = run(inputs, W, trace=False)
    return out
